# revision 6
# baseline (speedup 1.0000x reference)
"""Trainium2 Bass kernel for nn_BlockDecomposition (relational GNN message passing).

out[n] = sum_r sum_{e: type=r, tgt=n} w_e * (x[src_e] @ BD(blocks[r]))

v3: targets -> cores; 64-target windows, all 8 relations folded into one
512-column one-hot (column = enc(r*64 + t_off); enc keeps values bf16-exact).
Per window (9 chunks of 128 edges):
  - dma_gather x[src] bf16 tokens (256B, duplicated row) from DRAM,
  - one batched DVE multiply scales each gathered tile by edge weights
    (broadcast AP over the 64 feature columns),
  - ONE batched DVE tensor_tensor is_equal builds all 9 one-hots
    [128, 9, 512] against a broadcast iota (per-op DVE overhead amortized),
  - PE aggregation aggT[d, (r,t)] += msgs^T @ oh into one PSUM bank,
  - 8 accumulating W matmuls -> out2T[d, t], ACT eviction, staged DMA.
Edge streams use OVERLAPPING lo/hi source ranges (rows [0,32768) and
[N_PAD-32768, N_PAD)) with flexible per-bucket assignment of the overlap,
minimizing chunk padding while keeping one SPMD program (int16 idx limit).
Host: concatenate the 8 per-core transposed [64, 6272] slabs.
"""
import numpy as np

N_NODES = 50000
N_PAD = 50048
P = 128
D = 64
R = 8
TW = 64                  # targets per window
WPC = 98                 # windows per core
SLAB = WPC * TW          # 6272 targets per core
NCOL = R * TW            # 512 one-hot columns
LO_END = 32768
HI_START = N_PAD - 32768 # 17280
BATCH_CH = 32            # chunks per dma_gather (4096 indices)
EVW = 16                 # windows per eviction stage

_cache = {}


def _enc(c):
    """Injective map 0..511 -> bf16-exactly-representable integers."""
    c = np.asarray(c, np.int64)
    return np.where(c < 256, c, np.where(c < 384, 2 * c - 256, 4 * c - 1024)
                    ).astype(np.float32)


def _build_program(c_lo, c_hi):
    import concourse.bacc as bacc
    import concourse.tile as tile
    import concourse.mybir as mybir

    cpw = c_lo + c_hi
    nch_lo = WPC * c_lo
    nch_hi = WPC * c_hi
    nch = WPC * cpw

    nc = bacc.Bacc("TRN2", target_bir_lowering=False, debug=False, num_devices=8,
                   num_swdge_queues=4)

    xd_d = nc.dram_tensor("xd", [N_PAD, 2 * D], mybir.dt.bfloat16, kind="ExternalInput")
    il_d = nc.dram_tensor("il", [P, nch_lo * 8], mybir.dt.int16, kind="ExternalInput")
    ih_d = nc.dram_tensor("ih", [P, nch_hi * 8], mybir.dt.int16, kind="ExternalInput")
    wlo_d = nc.dram_tensor("wlo", [P, nch_lo], mybir.dt.bfloat16, kind="ExternalInput")
    whi_d = nc.dram_tensor("whi", [P, nch_hi], mybir.dt.bfloat16, kind="ExternalInput")
    tgt_d = nc.dram_tensor("tgt", [P, nch], mybir.dt.bfloat16, kind="ExternalInput")
    iota_d = nc.dram_tensor("iota", [P, NCOL], mybir.dt.bfloat16, kind="ExternalInput")
    wtab_d = nc.dram_tensor("wtab", [D, R * D], mybir.dt.bfloat16, kind="ExternalInput")
    out_d = nc.dram_tensor("outT", [D, SLAB], mybir.dt.float32, kind="ExternalOutput")

    with tile.TileContext(nc) as tc:
        with (
            tc.tile_pool(name="consts", bufs=1) as consts,
            tc.tile_pool(name="edges", bufs=1) as edges,
            tc.tile_pool(name="msgs", bufs=8) as msgs_pool,
            tc.tile_pool(name="oh", bufs=4) as oh_pool,
            tc.tile_pool(name="agg", bufs=3, space="PSUM") as agg_pool,
            tc.tile_pool(name="absb", bufs=3) as absb_pool,
            tc.tile_pool(name="out2", bufs=2, space="PSUM") as out2_pool,
            tc.tile_pool(name="evict", bufs=2) as evict_pool,
        ):
            iota_t = consts.tile([P, NCOL], mybir.dt.bfloat16, tag="iota")
            nc.sync.dma_start(iota_t[:], iota_d[:])
            wtab_t = consts.tile([D, R * D], mybir.dt.bfloat16, tag="wtab")
            nc.sync.dma_start(wtab_t[:], wtab_d[:])

            il_t = edges.tile([P, nch_lo * 8], mybir.dt.int16, tag="il")
            ih_t = edges.tile([P, nch_hi * 8], mybir.dt.int16, tag="ih")
            wlo_t = edges.tile([P, nch_lo], mybir.dt.bfloat16, tag="wlo")
            whi_t = edges.tile([P, nch_hi], mybir.dt.bfloat16, tag="whi")
            tgt_t = edges.tile([P, nch], mybir.dt.bfloat16, tag="tgt")
            nc.sync.dma_start(il_t[:], il_d[:])
            nc.sync.dma_start(ih_t[:], ih_d[:])
            nc.sync.dma_start(wlo_t[:], wlo_d[:])
            nc.sync.dma_start(whi_t[:], whi_d[:])
            nc.sync.dma_start(tgt_t[:], tgt_d[:])

            x_lo = xd_d[0:LO_END, :]
            x_hi = xd_d[HI_START:N_PAD, :]

            qrr = [0]

            def emit_gather(b, nch_s, idx_tile, w_tile, src_ap, tag):
                ch = min(BATCH_CH, nch_s - b * BATCH_CH)
                ni = ch * P
                mt = msgs_pool.tile([P, BATCH_CH * 2 * D], mybir.dt.bfloat16, tag=tag)
                nc.gpsimd.dma_gather(
                    out_ap=mt[:, :ch * 2 * D].rearrange("p (c e) -> p c e", e=2 * D),
                    in_ap=src_ap,
                    idxs_ap=idx_tile[:, b * BATCH_CH * 8:b * BATCH_CH * 8 + ch * 8],
                    num_idxs=ni, num_idxs_reg=ni, elem_size=2 * D,
                    single_packet=False, queue_num=qrr[0] % 4)
                qrr[0] += 1
                # scale messages by edge weights (first D cols of each token)
                mv = mt[:, :ch * 2 * D].rearrange("p (c e) -> p c e", e=2 * D)[:, :, 0:D]
                wv = w_tile[:, b * BATCH_CH:b * BATCH_CH + ch].unsqueeze(2) \
                    .to_broadcast((P, ch, D))
                nc.vector.tensor_tensor(out=mv, in0=mv, in1=wv,
                                        op=mybir.AluOpType.mult)
                return mt

            nb_lo = (nch_lo + BATCH_CH - 1) // BATCH_CH
            nb_hi = (nch_hi + BATCH_CH - 1) // BATCH_CH
            ev = []
            for b in range(nb_lo):
                ev.append((b * BATCH_CH // c_lo, 0, b))
            for b in range(nb_hi):
                ev.append((b * BATCH_CH // c_hi, 1, b))
            ev.sort()
            lo_tiles, hi_tiles = {}, {}
            for _, s, b in ev:
                if s == 0:
                    lo_tiles[b] = emit_gather(b, nch_lo, il_t, wlo_t, x_lo, "mlo")
                else:
                    hi_tiles[b] = emit_gather(b, nch_hi, ih_t, whi_t, x_hi, "mhi")

            stg = None
            for w in range(WPC):
                # batched one-hot: all cpw chunks of the window in one DVE op
                ohw = oh_pool.tile([P, cpw * NCOL], mybir.dt.bfloat16, tag="ohw")
                oh3 = ohw[:].rearrange("p (k q) -> p k q", q=NCOL)
                nc.vector.tensor_tensor(
                    out=oh3,
                    in0=iota_t[:].unsqueeze(1).to_broadcast((P, cpw, NCOL)),
                    in1=tgt_t[:, w * cpw:(w + 1) * cpw].unsqueeze(2)
                        .to_broadcast((P, cpw, NCOL)),
                    op=mybir.AluOpType.is_equal)
                ps = agg_pool.tile([D, NCOL], mybir.dt.float32, space="PSUM", tag="agg")
                for c in range(cpw):
                    if c < c_lo:
                        js = w * c_lo + c
                        mt = lo_tiles[js // BATCH_CH]
                    else:
                        js = w * c_hi + (c - c_lo)
                        mt = hi_tiles[js // BATCH_CH]
                    jl = js % BATCH_CH
                    nc.tensor.matmul(
                        out=ps[:], lhsT=mt[:, jl * 2 * D:jl * 2 * D + D],
                        rhs=ohw[:, c * NCOL:(c + 1) * NCOL],
                        start=(c == 0), stop=(c == cpw - 1))
                ab = absb_pool.tile([D, NCOL], mybir.dt.bfloat16, tag="ab")
                nc.scalar.copy(ab[:], ps[:])
                o2 = out2_pool.tile([D, TW], mybir.dt.float32, space="PSUM", tag="o2")
                for r in range(R):
                    nc.tensor.matmul(
                        out=o2[:], lhsT=wtab_t[:, r * D:(r + 1) * D],
                        rhs=ab[:, r * TW:(r + 1) * TW],
                        start=(r == 0), stop=(r == R - 1))
                si = w % EVW
                if si == 0:
                    stg = evict_pool.tile([D, EVW * TW], mybir.dt.float32, tag="stg")
                nc.scalar.copy(stg[:, si * TW:(si + 1) * TW], o2[:])
                if si == EVW - 1 or w == WPC - 1:
                    w0 = w - si
                    nc.sync.dma_start(
                        out_d[:, w0 * TW:(w + 1) * TW], stg[:, :(si + 1) * TW])

    nc.compile()
    return nc


def _prep_core(src, bidx, col, wgt, c_lo, c_hi):
    """Build per-core edge streams; edges sorted by bucket. src = node index,
    col = encoded (r, t_off) column value (f32). Flexible lo/hi assignment.
    Returns il, ih (wrapped idx), wlo, whi (stream-order weights), tgt_arr."""
    cpw = c_lo + c_hi
    nch_lo = WPC * c_lo
    nch_hi = WPC * c_hi
    nch = WPC * cpw
    cap_lo = c_lo * P
    cap_hi = c_hi * P

    il = np.zeros(nch_lo * P, np.int16)
    ih = np.zeros(nch_hi * P, np.int16)
    wlo = np.zeros((P, nch_lo), np.float32)
    whi = np.zeros((P, nch_hi), np.float32)
    tgt_arr = np.zeros((P, nch), np.float32)

    starts = np.searchsorted(bidx, np.arange(WPC + 1))
    for b in range(WPC):
        s0, s1 = starts[b], starts[b + 1]
        sb, cb, wb = src[s0:s1], col[s0:s1], wgt[s0:s1]
        T = s1 - s0
        must_lo = sb < HI_START
        mid = ~must_lo & (sb < LO_END)
        n_ml = int(must_lo.sum())
        lo_target = max(n_ml, T - cap_hi)
        take = lo_target - n_ml
        lo_sel = must_lo.copy()
        if take > 0:
            mi = np.flatnonzero(mid)[:take]
            lo_sel[mi] = True
        hi_sel = ~lo_sel
        n_lo = int(lo_sel.sum())
        n_hi = T - n_lo
        assert n_lo <= cap_lo and n_hi <= cap_hi, (b, n_lo, n_hi)
        for sel, c_n, stream, warr, base, j0 in (
                (lo_sel, c_lo, il, wlo, 0, 0),
                (hi_sel, c_hi, ih, whi, HI_START, c_lo)):
            n = int(sel.sum())
            cap = c_n * P
            pos0 = b * cap
            stream[pos0:pos0 + n] = (sb[sel] - base).astype(np.int16)
            slots = np.arange(cap)
            cw = slots // P
            lane = slots % P
            js = b * c_n + cw                 # stream-order chunk
            j = b * cpw + j0 + cw             # consumption-order chunk
            wcol = np.zeros(cap, np.float32)
            tcol = np.zeros(cap, np.float32)
            wcol[:n] = wb[sel]
            tcol[:n] = cb[sel]
            warr[lane, js] = wcol
            tgt_arr[lane, j] = tcol

    def wrap(stream, nch_s):
        out = np.zeros((P, nch_s * 8), np.int16)
        nbt = (nch_s + BATCH_CH - 1) // BATCH_CH
        for b in range(nbt):
            ch = min(BATCH_CH, nch_s - b * BATCH_CH)
            seg = stream[b * BATCH_CH * P: b * BATCH_CH * P + ch * P]
            w16 = seg.reshape(ch * 8, 16).T
            out[:, b * BATCH_CH * 8: b * BATCH_CH * 8 + ch * 8] = np.tile(w16, (8, 1))
        return out

    return wrap(il, nch_lo), wrap(ih, nch_hi), wlo, whi, tgt_arr


def _bf16(a):
    import ml_dtypes
    return a.astype(ml_dtypes.bfloat16)


def kernel(x, blocks, edge_weights, source, target, edge_type):
    from concourse.bass_utils import run_bass_kernel_spmd

    x = np.asarray(x, np.float32)
    blocks = np.asarray(blocks, np.float32)
    edge_weights = np.asarray(edge_weights, np.float32)
    source = np.asarray(source, np.int64)
    target = np.asarray(target, np.int64)
    edge_type = np.asarray(edge_type, np.int64)

    n, d = x.shape
    assert n == N_NODES and d == D

    xp = np.zeros((N_PAD, 2 * D), np.float32)
    xp[:n, :D] = x
    xp[:n, D:] = x
    xd = _bf16(xp)

    iota = _bf16(np.broadcast_to(_enc(np.arange(NCOL)), (P, NCOL)).copy())

    bs = D // blocks.shape[1]
    wtab = np.zeros((D, R * D), np.float32)
    for r in range(R):
        for b in range(blocks.shape[1]):
            wtab[b * bs:(b + 1) * bs, r * D + b * bs:r * D + (b + 1) * bs] = blocks[r, b]
    wtab = _bf16(wtab)

    core = np.minimum(target // SLAB, R - 1)
    per_core = []
    c_tot_min = 1
    max_ml = max_mh = 0
    for c in range(R):
        m = core == c
        src_c = source[m]
        bidx = (target[m] - c * SLAB) // TW
        col = _enc(edge_type[m] * TW + target[m] % TW)
        wgt_c = edge_weights[m]
        order = np.argsort(bidx, kind="stable")
        src_c, bidx, col, wgt_c = (a[order] for a in (src_c, bidx, col, wgt_c))
        T = np.bincount(bidx, minlength=WPC)
        ml = np.bincount(bidx[src_c < HI_START], minlength=WPC)
        mh = np.bincount(bidx[src_c >= LO_END], minlength=WPC)
        c_tot_min = max(c_tot_min, int(-(-T.max() // P)))
        max_ml = max(max_ml, int(ml.max()))
        max_mh = max(max_mh, int(mh.max()))
        per_core.append((src_c, bidx, col, wgt_c))

    c_lo = max(1, int(-(-max_ml // P)))
    c_hi = max(1, c_tot_min - c_lo)
    while c_hi * P < max_mh:
        c_hi += 1

    key = (c_lo, c_hi)
    if key not in _cache:
        _cache[key] = _build_program(c_lo, c_hi)
    nc = _cache[key]

    in_maps = []
    for c in range(R):
        src_c, bidx, col, wgt_c = per_core[c]
        il, ih, wlo, whi, tgt_arr = _prep_core(src_c, bidx, col, wgt_c, c_lo, c_hi)
        in_maps.append({
            "xd": xd, "il": il, "ih": ih,
            "wlo": _bf16(wlo), "whi": _bf16(whi), "tgt": _bf16(tgt_arr),
            "iota": iota, "wtab": wtab,
        })

    res = run_bass_kernel_spmd(nc, in_maps, core_ids=list(range(R)))

    out = np.zeros((R * SLAB, D), np.float32)
    for c in range(R):
        out[c * SLAB:(c + 1) * SLAB] = res.results[c]["outT"].T
    return out[:N_NODES]


# revision 8
# speedup vs baseline: 1.1088x; 1.1088x over previous
"""Trainium2 Bass kernel for nn_BlockDecomposition (relational GNN message passing).

out[n] = sum_r sum_{e: type=r, tgt=n} w_e * (x[src_e] @ BD(blocks[r]))

v4: targets -> cores with HOST-SIDE TARGET RELABELING: targets are packed
into 64-target windows balanced by degree (greedy bin-packing), so every
window holds <= 1024 edges = 8 chunks with ~zero padding (100352 gather
rows/core vs 150528 unpacked). All 8 relations fold into one 512-column
one-hot (column = enc(r*64 + t_off), enc keeps values bf16-exact).

Engine split (the point of this revision — no cross-engine head-of-line
stalls): DVE builds ONLY the batched is_equal one-hots ([128, 8, 512] per
window, constant inputs, runs ahead freely); the Scalar engine applies the
per-edge weights to gathered messages (per-chunk activation Copy with the
weight column as per-partition scale); PE aggregates 8 matmuls/window into
a [64, 512] PSUM bank and applies W via window-pair-batched matmuls
(strided rhs AP halves LDWEIGHTS count). Gathers run on the GpSimd SWDGE
queue (the critical resource, ~4ns/row descriptor generation).

Edge streams use OVERLAPPING lo/hi source ranges (rows [0,32768) and
[N_PAD-32768, N_PAD)) with flexible per-bucket assignment of the overlap.
Host: concatenate per-core [64, 6272] slabs, invert the target relabeling.
"""
import numpy as np

N_NODES = 50000
N_PAD = 50048
P = 128
D = 64
R = 8
TW = 64                  # targets per window
WPC = 98                 # windows per core
NW = N_PAD // TW         # 782 real windows
SLAB = WPC * TW          # 6272 targets per core
NCOL = R * TW            # 512 one-hot columns
LO_END = 32768
HI_START = N_PAD - 32768 # 17280
BATCH_CH = 49            # chunks per dma_gather (6272 indices)
WPAIR = 2                # windows per W-matmul group
EVG = 4                  # pair-groups per eviction stage

_cache = {}


def _enc(c):
    """Injective map 0..511 -> bf16-exactly-representable integers."""
    c = np.asarray(c, np.int64)
    return np.where(c < 256, c, np.where(c < 384, 2 * c - 256, 4 * c - 1024)
                    ).astype(np.float32)


def _build_program(c_lo, c_hi):
    import concourse.bacc as bacc
    import concourse.tile as tile
    import concourse.mybir as mybir

    cpw = c_lo + c_hi
    nch_lo = WPC * c_lo
    nch_hi = WPC * c_hi
    nch = WPC * cpw

    nc = bacc.Bacc("TRN2", target_bir_lowering=False, debug=False, num_devices=8,
                   num_swdge_queues=4)

    xd_d = nc.dram_tensor("xd", [N_PAD, 2 * D], mybir.dt.bfloat16, kind="ExternalInput")
    il_d = nc.dram_tensor("il", [P, nch_lo * 8], mybir.dt.int16, kind="ExternalInput")
    ih_d = nc.dram_tensor("ih", [P, nch_hi * 8], mybir.dt.int16, kind="ExternalInput")
    wlo_d = nc.dram_tensor("wlo", [P, nch_lo], mybir.dt.float32, kind="ExternalInput")
    whi_d = nc.dram_tensor("whi", [P, nch_hi], mybir.dt.float32, kind="ExternalInput")
    tgt_d = nc.dram_tensor("tgt", [P, nch], mybir.dt.bfloat16, kind="ExternalInput")
    iota_d = nc.dram_tensor("iota", [P, NCOL], mybir.dt.bfloat16, kind="ExternalInput")
    wtab_d = nc.dram_tensor("wtab", [D, R * D], mybir.dt.bfloat16, kind="ExternalInput")
    out_d = nc.dram_tensor("outT", [D, SLAB], mybir.dt.float32, kind="ExternalOutput")

    with tile.TileContext(nc) as tc:
        with (
            tc.tile_pool(name="consts", bufs=1) as consts,
            tc.tile_pool(name="edges", bufs=1) as edges,
            tc.tile_pool(name="msgs", bufs=5) as msgs_pool,
            tc.tile_pool(name="oh", bufs=3) as oh_pool,
            tc.tile_pool(name="agg", bufs=3, space="PSUM") as agg_pool,
            tc.tile_pool(name="absb", bufs=3) as absb_pool,
            tc.tile_pool(name="out2", bufs=2, space="PSUM") as out2_pool,
            tc.tile_pool(name="evict", bufs=2) as evict_pool,
        ):
            iota_t = consts.tile([P, NCOL], mybir.dt.bfloat16, tag="iota")
            nc.sync.dma_start(iota_t[:], iota_d[:])
            wtab_t = consts.tile([D, R * D], mybir.dt.bfloat16, tag="wtab")
            nc.sync.dma_start(wtab_t[:], wtab_d[:])

            il_t = edges.tile([P, nch_lo * 8], mybir.dt.int16, tag="il")
            ih_t = edges.tile([P, nch_hi * 8], mybir.dt.int16, tag="ih")
            wlo_t = edges.tile([P, nch_lo], mybir.dt.float32, tag="wlo")
            whi_t = edges.tile([P, nch_hi], mybir.dt.float32, tag="whi")
            tgt_t = edges.tile([P, nch], mybir.dt.bfloat16, tag="tgt")
            nc.sync.dma_start(il_t[:], il_d[:])
            nc.sync.dma_start(ih_t[:], ih_d[:])
            nc.sync.dma_start(wlo_t[:], wlo_d[:])
            nc.sync.dma_start(whi_t[:], whi_d[:])
            nc.sync.dma_start(tgt_t[:], tgt_d[:])

            x_lo = xd_d[0:LO_END, :]
            x_hi = xd_d[HI_START:N_PAD, :]

            qrr = [0]

            def emit_gather(b, nch_s, idx_tile, w_tile, src_ap, tag):
                ch = min(BATCH_CH, nch_s - b * BATCH_CH)
                ni = ch * P
                mt = msgs_pool.tile([P, BATCH_CH * 2 * D], mybir.dt.bfloat16, tag=tag)
                nc.gpsimd.dma_gather(
                    out_ap=mt[:, :ch * 2 * D].rearrange("p (c e) -> p c e", e=2 * D),
                    in_ap=src_ap,
                    idxs_ap=idx_tile[:, b * BATCH_CH * 8:b * BATCH_CH * 8 + ch * 8],
                    num_idxs=ni, num_idxs_reg=ni, elem_size=2 * D,
                    single_packet=False, queue_num=qrr[0] % 4)
                qrr[0] += 1
                # apply edge weights on the Scalar engine: per-chunk copy with
                # the weight column as per-partition scale (in place, first D
                # columns of each 2D-wide token)
                for k in range(ch):
                    js = b * BATCH_CH + k
                    mv = mt[:, k * 2 * D:k * 2 * D + D]
                    nc.scalar.activation(
                        mv, mv, mybir.ActivationFunctionType.Copy,
                        scale=w_tile[:, js:js + 1])
                return mt

            nb_lo = (nch_lo + BATCH_CH - 1) // BATCH_CH
            nb_hi = (nch_hi + BATCH_CH - 1) // BATCH_CH
            ev = []
            for b in range(nb_lo):
                ev.append((b * BATCH_CH // c_lo, 0, b))
            for b in range(nb_hi):
                ev.append((b * BATCH_CH // c_hi, 1, b))
            ev.sort()
            lo_tiles, hi_tiles = {}, {}
            for _, s, b in ev:
                if s == 0:
                    lo_tiles[b] = emit_gather(b, nch_lo, il_t, wlo_t, x_lo, "mlo")
                else:
                    hi_tiles[b] = emit_gather(b, nch_hi, ih_t, whi_t, x_hi, "mhi")

            stg = None
            ab2 = None
            for w in range(WPC):
                ohw = oh_pool.tile([P, cpw * NCOL], mybir.dt.bfloat16, tag="ohw")
                nc.vector.tensor_tensor(
                    out=ohw[:].rearrange("p (k q) -> p k q", q=NCOL),
                    in0=iota_t[:].unsqueeze(1).to_broadcast((P, cpw, NCOL)),
                    in1=tgt_t[:, w * cpw:(w + 1) * cpw].unsqueeze(2)
                        .to_broadcast((P, cpw, NCOL)),
                    op=mybir.AluOpType.is_equal)
                ps = agg_pool.tile([D, NCOL], mybir.dt.float32, space="PSUM", tag="agg")
                for c in range(cpw):
                    if c < c_lo:
                        js = w * c_lo + c
                        mt = lo_tiles[js // BATCH_CH]
                    else:
                        js = w * c_hi + (c - c_lo)
                        mt = hi_tiles[js // BATCH_CH]
                    jl = js % BATCH_CH
                    nc.tensor.matmul(
                        out=ps[:], lhsT=mt[:, jl * 2 * D:jl * 2 * D + D],
                        rhs=ohw[:, c * NCOL:(c + 1) * NCOL],
                        start=(c == 0), stop=(c == cpw - 1))
                pi = w % WPAIR
                if pi == 0:
                    ab2 = absb_pool.tile([D, WPAIR * NCOL], mybir.dt.bfloat16, tag="ab")
                nc.scalar.copy(ab2[:, pi * NCOL:(pi + 1) * NCOL], ps[:])
                if pi != WPAIR - 1:
                    continue
                # one W pass for the window pair: rhs [64, WPAIR, TW] strided
                o2 = out2_pool.tile([D, WPAIR * TW], mybir.dt.float32, space="PSUM",
                                    tag="o2")
                ab3 = ab2[:].rearrange("p (k q) -> p k q", q=NCOL)
                for r in range(R):
                    nc.tensor.matmul(
                        out=o2[:], lhsT=wtab_t[:, r * D:(r + 1) * D],
                        rhs=ab3[:, :, r * TW:(r + 1) * TW],
                        start=(r == 0), stop=(r == R - 1))
                g = w // WPAIR
                si = g % EVG
                if si == 0:
                    stg = evict_pool.tile([D, EVG * WPAIR * TW], mybir.dt.float32,
                                          tag="stg")
                nc.scalar.copy(stg[:, si * WPAIR * TW:(si + 1) * WPAIR * TW], o2[:])
                if si == EVG - 1 or w == WPC - 1:
                    g0 = g - si
                    nc.sync.dma_start(
                        out_d[:, g0 * WPAIR * TW:(g + 1) * WPAIR * TW],
                        stg[:, :(si + 1) * WPAIR * TW])

    nc.compile()
    return nc


def _pack_windows(deg):
    """Greedy degree-balanced assignment of targets to 64-target windows.
    Returns (win_of, off_of) for all N_PAD targets."""
    import heapq
    order = np.argsort(-deg, kind="stable")
    win_of = np.zeros(N_PAD, np.int32)
    off_of = np.zeros(N_PAD, np.int32)
    sums = np.zeros(NW, np.int64)
    counts = np.zeros(NW, np.int32)
    heap = [(0, 0, w) for w in range(NW)]
    heapq.heapify(heap)
    for t in order:
        while True:
            s, cnt, w = heapq.heappop(heap)
            if counts[w] < TW and s == sums[w]:
                break
        win_of[t] = w
        off_of[t] = counts[w]
        counts[w] += 1
        sums[w] += deg[t]
        if counts[w] < TW:
            heapq.heappush(heap, (sums[w], counts[w], w))
    return win_of, off_of


def _prep_core(src, bidx, col, wgt, c_lo, c_hi):
    """Build per-core edge streams; edges sorted by bucket. Flexible lo/hi
    assignment. Returns il, ih (wrapped idx), wlo, whi (stream-order
    weights), tgt_arr (consumption-order encoded columns)."""
    cpw = c_lo + c_hi
    nch_lo = WPC * c_lo
    nch_hi = WPC * c_hi
    nch = WPC * cpw
    cap_lo = c_lo * P
    cap_hi = c_hi * P

    il = np.zeros(nch_lo * P, np.int16)
    ih = np.zeros(nch_hi * P, np.int16)
    wlo = np.zeros((P, nch_lo), np.float32)
    whi = np.zeros((P, nch_hi), np.float32)
    tgt_arr = np.zeros((P, nch), np.float32)

    starts = np.searchsorted(bidx, np.arange(WPC + 1))
    for b in range(WPC):
        s0, s1 = starts[b], starts[b + 1]
        sb, cb, wb = src[s0:s1], col[s0:s1], wgt[s0:s1]
        T = s1 - s0
        must_lo = sb < HI_START
        mid = ~must_lo & (sb < LO_END)
        n_ml = int(must_lo.sum())
        lo_target = max(n_ml, T - cap_hi)
        take = lo_target - n_ml
        lo_sel = must_lo.copy()
        if take > 0:
            mi = np.flatnonzero(mid)[:take]
            lo_sel[mi] = True
        hi_sel = ~lo_sel
        n_lo = int(lo_sel.sum())
        n_hi = T - n_lo
        assert n_lo <= cap_lo and n_hi <= cap_hi, (b, n_lo, n_hi)
        for sel, c_n, stream, warr, base, j0 in (
                (lo_sel, c_lo, il, wlo, 0, 0),
                (hi_sel, c_hi, ih, whi, HI_START, c_lo)):
            n = int(sel.sum())
            cap = c_n * P
            pos0 = b * cap
            stream[pos0:pos0 + n] = (sb[sel] - base).astype(np.int16)
            slots = np.arange(cap)
            cw = slots // P
            lane = slots % P
            js = b * c_n + cw
            j = b * cpw + j0 + cw
            wcol = np.zeros(cap, np.float32)
            tcol = np.zeros(cap, np.float32)
            wcol[:n] = wb[sel]
            tcol[:n] = cb[sel]
            warr[lane, js] = wcol
            tgt_arr[lane, j] = tcol

    def wrap(stream, nch_s):
        out = np.zeros((P, nch_s * 8), np.int16)
        nbt = (nch_s + BATCH_CH - 1) // BATCH_CH
        for b in range(nbt):
            ch = min(BATCH_CH, nch_s - b * BATCH_CH)
            seg = stream[b * BATCH_CH * P: b * BATCH_CH * P + ch * P]
            w16 = seg.reshape(ch * 8, 16).T
            out[:, b * BATCH_CH * 8: b * BATCH_CH * 8 + ch * 8] = np.tile(w16, (8, 1))
        return out

    return wrap(il, nch_lo), wrap(ih, nch_hi), wlo, whi, tgt_arr


def _bf16(a):
    import ml_dtypes
    return a.astype(ml_dtypes.bfloat16)


def kernel(x, blocks, edge_weights, source, target, edge_type):
    from concourse.bass_utils import run_bass_kernel_spmd

    x = np.asarray(x, np.float32)
    blocks = np.asarray(blocks, np.float32)
    edge_weights = np.asarray(edge_weights, np.float32)
    source = np.asarray(source, np.int64)
    target = np.asarray(target, np.int64)
    edge_type = np.asarray(edge_type, np.int64)

    n, d = x.shape
    assert n == N_NODES and d == D

    xp = np.zeros((N_PAD, 2 * D), np.float32)
    xp[:n, :D] = x
    xp[:n, D:] = x
    xd = _bf16(xp)

    iota = _bf16(np.broadcast_to(_enc(np.arange(NCOL)), (P, NCOL)).copy())

    bs = D // blocks.shape[1]
    wtab = np.zeros((D, R * D), np.float32)
    for r in range(R):
        for b in range(blocks.shape[1]):
            wtab[b * bs:(b + 1) * bs, r * D + b * bs:r * D + (b + 1) * bs] = blocks[r, b]
    wtab = _bf16(wtab)

    deg = np.bincount(target, minlength=N_PAD)
    win_of, off_of = _pack_windows(deg)

    ewin = win_of[target]
    eoff = off_of[target]
    ecore = np.minimum(ewin // WPC, R - 1)
    per_core = []
    c_tot_min = 1
    max_ml = max_mh = 0
    for c in range(R):
        m = ecore == c
        src_c = source[m]
        bidx = (ewin[m] - c * WPC).astype(np.int64)
        col = _enc(edge_type[m] * TW + eoff[m])
        wgt_c = edge_weights[m]
        order = np.argsort(bidx, kind="stable")
        src_c, bidx, col, wgt_c = (a[order] for a in (src_c, bidx, col, wgt_c))
        T = np.bincount(bidx, minlength=WPC)
        ml = np.bincount(bidx[src_c < HI_START], minlength=WPC)
        mh = np.bincount(bidx[src_c >= LO_END], minlength=WPC)
        c_tot_min = max(c_tot_min, int(-(-T.max() // P)))
        max_ml = max(max_ml, int(ml.max()))
        max_mh = max(max_mh, int(mh.max()))
        per_core.append((src_c, bidx, col, wgt_c))

    c_lo = max(1, int(-(-max_ml // P)))
    c_hi = max(1, c_tot_min - c_lo)
    while c_hi * P < max_mh:
        c_hi += 1

    key = (c_lo, c_hi)
    if key not in _cache:
        _cache[key] = _build_program(c_lo, c_hi)
    nc = _cache[key]

    in_maps = []
    for c in range(R):
        src_c, bidx, col, wgt_c = per_core[c]
        il, ih, wlo, whi, tgt_arr = _prep_core(src_c, bidx, col, wgt_c, c_lo, c_hi)
        in_maps.append({
            "xd": xd, "il": il, "ih": ih,
            "wlo": wlo, "whi": whi, "tgt": _bf16(tgt_arr),
            "iota": iota, "wtab": wtab,
        })

    res = run_bass_kernel_spmd(nc, in_maps, core_ids=list(range(R)))

    out_perm = np.concatenate([res.results[c]["outT"] for c in range(R)], axis=1)
    t = np.arange(N_NODES)
    return np.ascontiguousarray(out_perm[:, win_of[t] * TW + off_of[t]].T)


# revision 9
# speedup vs baseline: 1.3664x; 1.2324x over previous
"""Trainium2 Bass kernel for nn_BlockDecomposition (relational GNN message passing).

out[n] = sum_r sum_{e: type=r, tgt=n} w_e * (x[src_e] @ BD(blocks[r]))

v4: targets -> cores with HOST-SIDE TARGET RELABELING: targets are packed
into 64-target windows balanced by degree (greedy bin-packing), so every
window holds <= 1024 edges = 8 chunks with ~zero padding (100352 gather
rows/core vs 150528 unpacked). All 8 relations fold into one 512-column
one-hot (column = enc(r*64 + t_off), enc keeps values bf16-exact).

Engine split (the point of this revision — no cross-engine head-of-line
stalls): DVE builds ONLY the batched is_equal one-hots ([128, 8, 512] per
window, constant inputs, runs ahead freely); the Scalar engine applies the
per-edge weights to gathered messages (per-chunk activation Copy with the
weight column as per-partition scale); PE aggregates 8 matmuls/window into
a [64, 512] PSUM bank and applies W via window-pair-batched matmuls
(strided rhs AP halves LDWEIGHTS count). Gathers run on the GpSimd SWDGE
queue (the critical resource, ~4ns/row descriptor generation).

Edge streams use OVERLAPPING lo/hi source ranges (rows [0,32768) and
[N_PAD-32768, N_PAD)) with flexible per-bucket assignment of the overlap.
Host: concatenate per-core [64, 6272] slabs, invert the target relabeling.
"""
import numpy as np

N_NODES = 50000
N_PAD = 50048
P = 128
D = 64
R = 8
TW = 64                  # targets per window
WPC = 98                 # windows per core
NW = N_PAD // TW         # 782 real windows
SLAB = WPC * TW          # 6272 targets per core
NCOL = R * TW            # 512 one-hot columns
LO_END = 32768
HI_START = N_PAD - 32768 # 17280
BATCH_CH = 32            # chunks per dma_gather (4096 indices)
WPAIR = 2                # windows per W-matmul group
EVG = 4                  # pair-groups per eviction stage

_cache = {}


def _enc(c):
    """Injective map 0..511 -> bf16-exactly-representable integers."""
    c = np.asarray(c, np.int64)
    return np.where(c < 256, c, np.where(c < 384, 2 * c - 256, 4 * c - 1024)
                    ).astype(np.float32)


def _build_program(c_lo, c_hi):
    import concourse.bacc as bacc
    import concourse.tile as tile
    import concourse.mybir as mybir

    cpw = c_lo + c_hi
    nch_lo = WPC * c_lo
    nch_hi = WPC * c_hi
    nch = WPC * cpw

    nc = bacc.Bacc("TRN2", target_bir_lowering=False, debug=False, num_devices=8,
                   num_swdge_queues=4)

    xd_d = nc.dram_tensor("xd", [N_PAD, 2 * D], mybir.dt.bfloat16, kind="ExternalInput")
    il_d = nc.dram_tensor("il", [P, nch_lo * 8], mybir.dt.int16, kind="ExternalInput")
    ih_d = nc.dram_tensor("ih", [P, nch_hi * 8], mybir.dt.int16, kind="ExternalInput")
    wlo_d = nc.dram_tensor("wlo", [P, nch_lo], mybir.dt.bfloat16, kind="ExternalInput")
    whi_d = nc.dram_tensor("whi", [P, nch_hi], mybir.dt.bfloat16, kind="ExternalInput")
    tgt_d = nc.dram_tensor("tgt", [P, nch], mybir.dt.bfloat16, kind="ExternalInput")
    iota_d = nc.dram_tensor("iota", [P, NCOL], mybir.dt.bfloat16, kind="ExternalInput")
    wtab_d = nc.dram_tensor("wtab", [D, R * D], mybir.dt.bfloat16, kind="ExternalInput")
    out_d = nc.dram_tensor("outT", [D, SLAB], mybir.dt.float32, kind="ExternalOutput")

    with tile.TileContext(nc) as tc:
        with (
            tc.tile_pool(name="consts", bufs=1) as consts,
            tc.tile_pool(name="edges", bufs=1) as edges,
            tc.tile_pool(name="msgs", bufs=8) as msgs_pool,
            tc.tile_pool(name="oh", bufs=3) as oh_pool,
            tc.tile_pool(name="agg", bufs=3, space="PSUM") as agg_pool,
            tc.tile_pool(name="absb", bufs=3) as absb_pool,
            tc.tile_pool(name="out2", bufs=2, space="PSUM") as out2_pool,
            tc.tile_pool(name="evict", bufs=2) as evict_pool,
        ):
            iota_t = consts.tile([P, NCOL], mybir.dt.bfloat16, tag="iota")
            nc.sync.dma_start(iota_t[:], iota_d[:])
            wtab_t = consts.tile([D, R * D], mybir.dt.bfloat16, tag="wtab")
            nc.sync.dma_start(wtab_t[:], wtab_d[:])

            il_t = edges.tile([P, nch_lo * 8], mybir.dt.int16, tag="il")
            ih_t = edges.tile([P, nch_hi * 8], mybir.dt.int16, tag="ih")
            wlo_t = edges.tile([P, nch_lo], mybir.dt.bfloat16, tag="wlo")
            whi_t = edges.tile([P, nch_hi], mybir.dt.bfloat16, tag="whi")
            tgt_t = edges.tile([P, nch], mybir.dt.bfloat16, tag="tgt")
            nc.sync.dma_start(il_t[:], il_d[:])
            nc.sync.dma_start(ih_t[:], ih_d[:])
            nc.sync.dma_start(wlo_t[:], wlo_d[:])
            nc.sync.dma_start(whi_t[:], whi_d[:])
            nc.sync.dma_start(tgt_t[:], tgt_d[:])

            x_lo = xd_d[0:LO_END, :]
            x_hi = xd_d[HI_START:N_PAD, :]

            qrr = [0]

            def emit_gather(b, nch_s, idx_tile, w_tile, src_ap, tag):
                ch = min(BATCH_CH, nch_s - b * BATCH_CH)
                ni = ch * P
                mt = msgs_pool.tile([P, BATCH_CH * 2 * D], mybir.dt.bfloat16, tag=tag)
                nc.gpsimd.dma_gather(
                    out_ap=mt[:, :ch * 2 * D].rearrange("p (c e) -> p c e", e=2 * D),
                    in_ap=src_ap,
                    idxs_ap=idx_tile[:, b * BATCH_CH * 8:b * BATCH_CH * 8 + ch * 8],
                    num_idxs=ni, num_idxs_reg=ni, elem_size=2 * D,
                    single_packet=False, queue_num=qrr[0] % 4)
                qrr[0] += 1
                # apply edge weights: one batched DVE multiply per gather call
                # (in place, first D columns of each 2D-wide token)
                mv = mt[:, :ch * 2 * D].rearrange("p (c e) -> p c e", e=2 * D)[:, :, 0:D]
                wv = w_tile[:, b * BATCH_CH:b * BATCH_CH + ch].unsqueeze(2) \
                    .to_broadcast((P, ch, D))
                nc.vector.tensor_tensor(out=mv, in0=mv, in1=wv,
                                        op=mybir.AluOpType.mult)
                return mt

            nb_lo = (nch_lo + BATCH_CH - 1) // BATCH_CH
            nb_hi = (nch_hi + BATCH_CH - 1) // BATCH_CH
            ev = []
            for b in range(nb_lo):
                ev.append((b * BATCH_CH // c_lo, 0, b))
            for b in range(nb_hi):
                ev.append((b * BATCH_CH // c_hi, 1, b))
            ev.sort()
            lo_tiles, hi_tiles = {}, {}
            for _, s, b in ev:
                if s == 0:
                    lo_tiles[b] = emit_gather(b, nch_lo, il_t, wlo_t, x_lo, "mlo")
                else:
                    hi_tiles[b] = emit_gather(b, nch_hi, ih_t, whi_t, x_hi, "mhi")

            stg = None
            ab2 = None
            for w in range(WPC):
                ohw = oh_pool.tile([P, cpw * NCOL], mybir.dt.bfloat16, tag="ohw")
                nc.vector.tensor_tensor(
                    out=ohw[:].rearrange("p (k q) -> p k q", q=NCOL),
                    in0=iota_t[:].unsqueeze(1).to_broadcast((P, cpw, NCOL)),
                    in1=tgt_t[:, w * cpw:(w + 1) * cpw].unsqueeze(2)
                        .to_broadcast((P, cpw, NCOL)),
                    op=mybir.AluOpType.is_equal)
                ps = agg_pool.tile([D, NCOL], mybir.dt.float32, space="PSUM", tag="agg")
                for c in range(cpw):
                    if c < c_lo:
                        js = w * c_lo + c
                        mt = lo_tiles[js // BATCH_CH]
                    else:
                        js = w * c_hi + (c - c_lo)
                        mt = hi_tiles[js // BATCH_CH]
                    jl = js % BATCH_CH
                    nc.tensor.matmul(
                        out=ps[:], lhsT=mt[:, jl * 2 * D:jl * 2 * D + D],
                        rhs=ohw[:, c * NCOL:(c + 1) * NCOL],
                        start=(c == 0), stop=(c == cpw - 1))
                pi = w % WPAIR
                if pi == 0:
                    ab2 = absb_pool.tile([D, WPAIR * NCOL], mybir.dt.bfloat16, tag="ab")
                nc.scalar.copy(ab2[:, pi * NCOL:(pi + 1) * NCOL], ps[:])
                if pi != WPAIR - 1:
                    continue
                # one W pass for the window pair: rhs [64, WPAIR, TW] strided
                o2 = out2_pool.tile([D, WPAIR * TW], mybir.dt.float32, space="PSUM",
                                    tag="o2")
                ab3 = ab2[:].rearrange("p (k q) -> p k q", q=NCOL)
                for r in range(R):
                    nc.tensor.matmul(
                        out=o2[:], lhsT=wtab_t[:, r * D:(r + 1) * D],
                        rhs=ab3[:, :, r * TW:(r + 1) * TW],
                        start=(r == 0), stop=(r == R - 1))
                g = w // WPAIR
                si = g % EVG
                if si == 0:
                    stg = evict_pool.tile([D, EVG * WPAIR * TW], mybir.dt.float32,
                                          tag="stg")
                nc.scalar.copy(stg[:, si * WPAIR * TW:(si + 1) * WPAIR * TW], o2[:])
                if si == EVG - 1 or w == WPC - 1:
                    g0 = g - si
                    nc.sync.dma_start(
                        out_d[:, g0 * WPAIR * TW:(g + 1) * WPAIR * TW],
                        stg[:, :(si + 1) * WPAIR * TW])

    nc.compile()
    return nc


def _pack_windows(deg):
    """Greedy degree-balanced assignment of targets to 64-target windows.
    Returns (win_of, off_of) for all N_PAD targets."""
    import heapq
    order = np.argsort(-deg, kind="stable")
    win_of = np.zeros(N_PAD, np.int32)
    off_of = np.zeros(N_PAD, np.int32)
    sums = np.zeros(NW, np.int64)
    counts = np.zeros(NW, np.int32)
    heap = [(0, 0, w) for w in range(NW)]
    heapq.heapify(heap)
    for t in order:
        while True:
            s, cnt, w = heapq.heappop(heap)
            if counts[w] < TW and s == sums[w]:
                break
        win_of[t] = w
        off_of[t] = counts[w]
        counts[w] += 1
        sums[w] += deg[t]
        if counts[w] < TW:
            heapq.heappush(heap, (sums[w], counts[w], w))
    return win_of, off_of


def _prep_core(src, bidx, col, wgt, c_lo, c_hi):
    """Build per-core edge streams; edges sorted by bucket. Flexible lo/hi
    assignment. Returns il, ih (wrapped idx), wlo, whi (stream-order
    weights), tgt_arr (consumption-order encoded columns)."""
    cpw = c_lo + c_hi
    nch_lo = WPC * c_lo
    nch_hi = WPC * c_hi
    nch = WPC * cpw
    cap_lo = c_lo * P
    cap_hi = c_hi * P

    il = np.zeros(nch_lo * P, np.int16)
    ih = np.zeros(nch_hi * P, np.int16)
    wlo = np.zeros((P, nch_lo), np.float32)
    whi = np.zeros((P, nch_hi), np.float32)
    tgt_arr = np.zeros((P, nch), np.float32)

    starts = np.searchsorted(bidx, np.arange(WPC + 1))
    for b in range(WPC):
        s0, s1 = starts[b], starts[b + 1]
        sb, cb, wb = src[s0:s1], col[s0:s1], wgt[s0:s1]
        T = s1 - s0
        must_lo = sb < HI_START
        mid = ~must_lo & (sb < LO_END)
        n_ml = int(must_lo.sum())
        lo_target = max(n_ml, T - cap_hi)
        take = lo_target - n_ml
        lo_sel = must_lo.copy()
        if take > 0:
            mi = np.flatnonzero(mid)[:take]
            lo_sel[mi] = True
        hi_sel = ~lo_sel
        n_lo = int(lo_sel.sum())
        n_hi = T - n_lo
        assert n_lo <= cap_lo and n_hi <= cap_hi, (b, n_lo, n_hi)
        for sel, c_n, stream, warr, base, j0 in (
                (lo_sel, c_lo, il, wlo, 0, 0),
                (hi_sel, c_hi, ih, whi, HI_START, c_lo)):
            n = int(sel.sum())
            cap = c_n * P
            pos0 = b * cap
            stream[pos0:pos0 + n] = (sb[sel] - base).astype(np.int16)
            slots = np.arange(cap)
            cw = slots // P
            lane = slots % P
            js = b * c_n + cw
            j = b * cpw + j0 + cw
            wcol = np.zeros(cap, np.float32)
            tcol = np.zeros(cap, np.float32)
            wcol[:n] = wb[sel]
            tcol[:n] = cb[sel]
            warr[lane, js] = wcol
            tgt_arr[lane, j] = tcol

    def wrap(stream, nch_s):
        out = np.zeros((P, nch_s * 8), np.int16)
        nbt = (nch_s + BATCH_CH - 1) // BATCH_CH
        for b in range(nbt):
            ch = min(BATCH_CH, nch_s - b * BATCH_CH)
            seg = stream[b * BATCH_CH * P: b * BATCH_CH * P + ch * P]
            w16 = seg.reshape(ch * 8, 16).T
            out[:, b * BATCH_CH * 8: b * BATCH_CH * 8 + ch * 8] = np.tile(w16, (8, 1))
        return out

    return wrap(il, nch_lo), wrap(ih, nch_hi), wlo, whi, tgt_arr


def _bf16(a):
    import ml_dtypes
    return a.astype(ml_dtypes.bfloat16)


def kernel(x, blocks, edge_weights, source, target, edge_type):
    from concourse.bass_utils import run_bass_kernel_spmd

    x = np.asarray(x, np.float32)
    blocks = np.asarray(blocks, np.float32)
    edge_weights = np.asarray(edge_weights, np.float32)
    source = np.asarray(source, np.int64)
    target = np.asarray(target, np.int64)
    edge_type = np.asarray(edge_type, np.int64)

    n, d = x.shape
    assert n == N_NODES and d == D

    xp = np.zeros((N_PAD, 2 * D), np.float32)
    xp[:n, :D] = x
    xp[:n, D:] = x
    xd = _bf16(xp)

    iota = _bf16(np.broadcast_to(_enc(np.arange(NCOL)), (P, NCOL)).copy())

    bs = D // blocks.shape[1]
    wtab = np.zeros((D, R * D), np.float32)
    for r in range(R):
        for b in range(blocks.shape[1]):
            wtab[b * bs:(b + 1) * bs, r * D + b * bs:r * D + (b + 1) * bs] = blocks[r, b]
    wtab = _bf16(wtab)

    deg = np.bincount(target, minlength=N_PAD)
    win_of, off_of = _pack_windows(deg)

    ewin = win_of[target]
    eoff = off_of[target]
    ecore = np.minimum(ewin // WPC, R - 1)
    per_core = []
    c_tot_min = 1
    max_ml = max_mh = 0
    for c in range(R):
        m = ecore == c
        src_c = source[m]
        bidx = (ewin[m] - c * WPC).astype(np.int64)
        col = _enc(edge_type[m] * TW + eoff[m])
        wgt_c = edge_weights[m]
        order = np.argsort(bidx, kind="stable")
        src_c, bidx, col, wgt_c = (a[order] for a in (src_c, bidx, col, wgt_c))
        T = np.bincount(bidx, minlength=WPC)
        ml = np.bincount(bidx[src_c < HI_START], minlength=WPC)
        mh = np.bincount(bidx[src_c >= LO_END], minlength=WPC)
        c_tot_min = max(c_tot_min, int(-(-T.max() // P)))
        max_ml = max(max_ml, int(ml.max()))
        max_mh = max(max_mh, int(mh.max()))
        per_core.append((src_c, bidx, col, wgt_c))

    c_lo = max(1, int(-(-max_ml // P)))
    c_hi = max(1, c_tot_min - c_lo)
    while c_hi * P < max_mh:
        c_hi += 1

    key = (c_lo, c_hi)
    if key not in _cache:
        _cache[key] = _build_program(c_lo, c_hi)
    nc = _cache[key]

    in_maps = []
    for c in range(R):
        src_c, bidx, col, wgt_c = per_core[c]
        il, ih, wlo, whi, tgt_arr = _prep_core(src_c, bidx, col, wgt_c, c_lo, c_hi)
        in_maps.append({
            "xd": xd, "il": il, "ih": ih,
            "wlo": _bf16(wlo), "whi": _bf16(whi), "tgt": _bf16(tgt_arr),
            "iota": iota, "wtab": wtab,
        })

    res = run_bass_kernel_spmd(nc, in_maps, core_ids=list(range(R)))

    out_perm = np.concatenate([res.results[c]["outT"] for c in range(R)], axis=1)
    t = np.arange(N_NODES)
    return np.ascontiguousarray(out_perm[:, win_of[t] * TW + off_of[t]].T)


# revision 10
# speedup vs baseline: 1.6453x; 1.2041x over previous
"""Trainium2 Bass kernel for nn_BlockDecomposition (relational GNN message passing).

out[n] = sum_r sum_{e: type=r, tgt=n} w_e * (x[src_e] @ BD(blocks[r]))

v6: targets -> cores with HOST-SIDE TARGET RELABELING: targets are packed
into degree-balanced 32-target windows (greedy bin-packing), bounding every
window at ~513 edges = 5 chunks (125440 gather rows/core, ~1.25x padding).
All 8 relations fold into one 256-column one-hot (column = r*32 + t_off,
all values bf16-exact).

Pipeline (engine roles chosen to avoid in-order head-of-line stalls):
  - GpSimd: dma_gather of x[src] bf16 tokens (256B duplicated rows) — the
    critical resource (~2.4ns/row descriptor generation). All gathers are
    emitted up front so the SWDGE queue runs ahead, bounded by msgs bufs.
  - DVE: per-gather-tile weight multiply (emitted JUST-IN-TIME in the
    window loop so it never blocks the queue ahead of one-hot builds) and
    one batched is_equal tensor_tensor per window [128, 5, 256].
  - PE: 5 aggregation matmuls/window into a [64, 256] PSUM tile; block-W
    applied via 4-window-batched matmuls (strided rhs AP).
  - ACT: PSUM->SBUF copies and staged output eviction.
Edge streams use OVERLAPPING lo/hi source ranges (rows [0,32768) and
[N_PAD-32768, N_PAD)) with flexible per-bucket assignment of the overlap
(int16 gather index limit).
Host: concatenate per-core [64, 6272] slabs, invert the target relabeling.
"""
import numpy as np

N_NODES = 50000
N_PAD = 50048
P = 128
D = 64
R = 8
TW = 32                  # targets per window
WPC = 196                # windows per core
NW = N_PAD // TW         # 1564 real windows
SLAB = WPC * TW          # 6272 targets per core
NCOL = R * TW            # 256 one-hot columns
LO_END = 32768
HI_START = N_PAD - 32768 # 17280
BATCH_CH = 32            # chunks per dma_gather (4096 indices)
WGRP = 4                 # windows per W-matmul group
EVG = 4                  # groups per eviction stage

_cache = {}


def _build_program(c_lo, c_hi):
    import concourse.bacc as bacc
    import concourse.tile as tile
    import concourse.mybir as mybir

    cpw = c_lo + c_hi
    nch_lo = WPC * c_lo
    nch_hi = WPC * c_hi
    nch = WPC * cpw

    nc = bacc.Bacc("TRN2", target_bir_lowering=False, debug=False, num_devices=8,
                   num_swdge_queues=4)

    xd_d = nc.dram_tensor("xd", [N_PAD, 2 * D], mybir.dt.bfloat16, kind="ExternalInput")
    il_d = nc.dram_tensor("il", [P, nch_lo * 8], mybir.dt.int16, kind="ExternalInput")
    ih_d = nc.dram_tensor("ih", [P, nch_hi * 8], mybir.dt.int16, kind="ExternalInput")
    wlo_d = nc.dram_tensor("wlo", [P, nch_lo], mybir.dt.bfloat16, kind="ExternalInput")
    whi_d = nc.dram_tensor("whi", [P, nch_hi], mybir.dt.bfloat16, kind="ExternalInput")
    tgt_d = nc.dram_tensor("tgt", [P, nch], mybir.dt.bfloat16, kind="ExternalInput")
    iota_d = nc.dram_tensor("iota", [P, NCOL], mybir.dt.bfloat16, kind="ExternalInput")
    wtab_d = nc.dram_tensor("wtab", [D, R * D], mybir.dt.bfloat16, kind="ExternalInput")
    out_d = nc.dram_tensor("outT", [D, SLAB], mybir.dt.float32, kind="ExternalOutput")

    with tile.TileContext(nc) as tc:
        with (
            tc.tile_pool(name="consts", bufs=1) as consts,
            tc.tile_pool(name="edges", bufs=1) as edges,
            tc.tile_pool(name="msgs", bufs=10) as msgs_pool,
            tc.tile_pool(name="oh", bufs=4) as oh_pool,
            tc.tile_pool(name="agg", bufs=4, space="PSUM") as agg_pool,
            tc.tile_pool(name="absb", bufs=3) as absb_pool,
            tc.tile_pool(name="out2", bufs=2, space="PSUM") as out2_pool,
            tc.tile_pool(name="evict", bufs=2) as evict_pool,
        ):
            iota_t = consts.tile([P, NCOL], mybir.dt.bfloat16, tag="iota")
            nc.sync.dma_start(iota_t[:], iota_d[:])
            wtab_t = consts.tile([D, R * D], mybir.dt.bfloat16, tag="wtab")
            nc.sync.dma_start(wtab_t[:], wtab_d[:])

            il_t = edges.tile([P, nch_lo * 8], mybir.dt.int16, tag="il")
            ih_t = edges.tile([P, nch_hi * 8], mybir.dt.int16, tag="ih")
            wlo_t = edges.tile([P, nch_lo], mybir.dt.bfloat16, tag="wlo")
            whi_t = edges.tile([P, nch_hi], mybir.dt.bfloat16, tag="whi")
            tgt_t = edges.tile([P, nch], mybir.dt.bfloat16, tag="tgt")
            nc.sync.dma_start(il_t[:], il_d[:])
            nc.sync.dma_start(ih_t[:], ih_d[:])
            nc.sync.dma_start(wlo_t[:], wlo_d[:])
            nc.sync.dma_start(whi_t[:], whi_d[:])
            nc.sync.dma_start(tgt_t[:], tgt_d[:])

            x_lo = xd_d[0:LO_END, :]
            x_hi = xd_d[HI_START:N_PAD, :]

            qrr = [0]

            def emit_gather(b, nch_s, idx_tile, src_ap, tag):
                ch = min(BATCH_CH, nch_s - b * BATCH_CH)
                ni = ch * P
                mt = msgs_pool.tile([P, BATCH_CH * 2 * D], mybir.dt.bfloat16, tag=tag)
                nc.gpsimd.dma_gather(
                    out_ap=mt[:, :ch * 2 * D].rearrange("p (c e) -> p c e", e=2 * D),
                    in_ap=src_ap,
                    idxs_ap=idx_tile[:, b * BATCH_CH * 8:b * BATCH_CH * 8 + ch * 8],
                    num_idxs=ni, num_idxs_reg=ni, elem_size=2 * D,
                    single_packet=False, queue_num=qrr[0] % 4)
                qrr[0] += 1
                return mt, ch

            # gathers up front (SWDGE runs ahead, bounded by msgs bufs);
            # weight-scale TTs are emitted just-in-time in the window loop
            nb_lo = (nch_lo + BATCH_CH - 1) // BATCH_CH
            nb_hi = (nch_hi + BATCH_CH - 1) // BATCH_CH
            ev = []
            for b in range(nb_lo):
                ev.append((b * BATCH_CH // c_lo, 0, b))
            for b in range(nb_hi):
                ev.append((b * BATCH_CH // c_hi, 1, b))
            ev.sort()
            tiles = {}          # (stream, b) -> (mt, ch)
            for _, s, b in ev:
                if s == 0:
                    tiles[(0, b)] = emit_gather(b, nch_lo, il_t, x_lo, "mlo")
                else:
                    tiles[(1, b)] = emit_gather(b, nch_hi, ih_t, x_hi, "mhi")

            def emit_scale(s, b):
                mt, ch = tiles[(s, b)]
                w_tile = wlo_t if s == 0 else whi_t
                mv = mt[:, :ch * 2 * D].rearrange("p (c e) -> p c e", e=2 * D)[:, :, 0:D]
                wv = w_tile[:, b * BATCH_CH:b * BATCH_CH + ch].unsqueeze(2) \
                    .to_broadcast((P, ch, D))
                nc.vector.tensor_tensor(out=mv, in0=mv, in1=wv,
                                        op=mybir.AluOpType.mult)

            # scale(s, b) must precede the first window consuming batch b
            scale_at = {}       # window -> list of (stream, b)
            for b in range(nb_lo):
                scale_at.setdefault(b * BATCH_CH // c_lo, []).append((0, b))
            for b in range(nb_hi):
                scale_at.setdefault(b * BATCH_CH // c_hi, []).append((1, b))

            stg = None
            ab2 = None
            for w in range(WPC):
                for s, b in scale_at.get(w, ()):
                    emit_scale(s, b)
                ohw = oh_pool.tile([P, cpw * NCOL], mybir.dt.bfloat16, tag="ohw")
                nc.vector.tensor_tensor(
                    out=ohw[:].rearrange("p (k q) -> p k q", q=NCOL),
                    in0=iota_t[:].unsqueeze(1).to_broadcast((P, cpw, NCOL)),
                    in1=tgt_t[:, w * cpw:(w + 1) * cpw].unsqueeze(2)
                        .to_broadcast((P, cpw, NCOL)),
                    op=mybir.AluOpType.is_equal)
                ps = agg_pool.tile([D, NCOL], mybir.dt.float32, space="PSUM", tag="agg")
                for c in range(cpw):
                    if c < c_lo:
                        js = w * c_lo + c
                        mt, _ = tiles[(0, js // BATCH_CH)]
                    else:
                        js = w * c_hi + (c - c_lo)
                        mt, _ = tiles[(1, js // BATCH_CH)]
                    jl = js % BATCH_CH
                    nc.tensor.matmul(
                        out=ps[:], lhsT=mt[:, jl * 2 * D:jl * 2 * D + D],
                        rhs=ohw[:, c * NCOL:(c + 1) * NCOL],
                        start=(c == 0), stop=(c == cpw - 1))
                gi = w % WGRP
                if gi == 0:
                    ab2 = absb_pool.tile([D, WGRP * NCOL], mybir.dt.bfloat16, tag="ab")
                nc.scalar.copy(ab2[:, gi * NCOL:(gi + 1) * NCOL], ps[:])
                if gi != WGRP - 1:
                    continue
                # one W pass for the window group: rhs [64, WGRP, TW] strided
                o2 = out2_pool.tile([D, WGRP * TW], mybir.dt.float32, space="PSUM",
                                    tag="o2")
                ab3 = ab2[:].rearrange("p (k q) -> p k q", q=NCOL)
                for r in range(R):
                    nc.tensor.matmul(
                        out=o2[:], lhsT=wtab_t[:, r * D:(r + 1) * D],
                        rhs=ab3[:, :, r * TW:(r + 1) * TW],
                        start=(r == 0), stop=(r == R - 1))
                g = w // WGRP
                si = g % EVG
                if si == 0:
                    stg = evict_pool.tile([D, EVG * WGRP * TW], mybir.dt.float32,
                                          tag="stg")
                nc.scalar.copy(stg[:, si * WGRP * TW:(si + 1) * WGRP * TW], o2[:])
                if si == EVG - 1 or w == WPC - 1:
                    g0 = g - si
                    nc.sync.dma_start(
                        out_d[:, g0 * WGRP * TW:(g + 1) * WGRP * TW],
                        stg[:, :(si + 1) * WGRP * TW])

    nc.compile()
    return nc


def _pack_windows(deg):
    """Greedy degree-balanced assignment of targets to TW-target windows.
    Returns (win_of, off_of) for all N_PAD targets."""
    import heapq
    order = np.argsort(-deg, kind="stable")
    win_of = np.zeros(N_PAD, np.int32)
    off_of = np.zeros(N_PAD, np.int32)
    sums = np.zeros(NW, np.int64)
    counts = np.zeros(NW, np.int32)
    heap = [(0, 0, w) for w in range(NW)]
    heapq.heapify(heap)
    for t in order:
        while True:
            s, cnt, w = heapq.heappop(heap)
            if counts[w] < TW and s == sums[w]:
                break
        win_of[t] = w
        off_of[t] = counts[w]
        counts[w] += 1
        sums[w] += deg[t]
        if counts[w] < TW:
            heapq.heappush(heap, (sums[w], counts[w], w))
    return win_of, off_of


def _prep_core(src, bidx, col, wgt, c_lo, c_hi):
    """Build per-core edge streams; edges sorted by bucket. Flexible lo/hi
    assignment. Returns il, ih (wrapped idx), wlo, whi (stream-order
    weights), tgt_arr (consumption-order column values)."""
    cpw = c_lo + c_hi
    nch_lo = WPC * c_lo
    nch_hi = WPC * c_hi
    nch = WPC * cpw
    cap_lo = c_lo * P
    cap_hi = c_hi * P

    il = np.zeros(nch_lo * P, np.int16)
    ih = np.zeros(nch_hi * P, np.int16)
    wlo = np.zeros((P, nch_lo), np.float32)
    whi = np.zeros((P, nch_hi), np.float32)
    tgt_arr = np.zeros((P, nch), np.float32)

    starts = np.searchsorted(bidx, np.arange(WPC + 1))
    for b in range(WPC):
        s0, s1 = starts[b], starts[b + 1]
        sb, cb, wb = src[s0:s1], col[s0:s1], wgt[s0:s1]
        T = s1 - s0
        must_lo = sb < HI_START
        mid = ~must_lo & (sb < LO_END)
        n_ml = int(must_lo.sum())
        lo_target = max(n_ml, T - cap_hi)
        take = lo_target - n_ml
        lo_sel = must_lo.copy()
        if take > 0:
            mi = np.flatnonzero(mid)[:take]
            lo_sel[mi] = True
        hi_sel = ~lo_sel
        n_lo = int(lo_sel.sum())
        n_hi = T - n_lo
        assert n_lo <= cap_lo and n_hi <= cap_hi, (b, n_lo, n_hi)
        for sel, c_n, stream, warr, base, j0 in (
                (lo_sel, c_lo, il, wlo, 0, 0),
                (hi_sel, c_hi, ih, whi, HI_START, c_lo)):
            n = int(sel.sum())
            cap = c_n * P
            pos0 = b * cap
            stream[pos0:pos0 + n] = (sb[sel] - base).astype(np.int16)
            slots = np.arange(cap)
            cw = slots // P
            lane = slots % P
            js = b * c_n + cw
            j = b * cpw + j0 + cw
            wcol = np.zeros(cap, np.float32)
            tcol = np.zeros(cap, np.float32)
            wcol[:n] = wb[sel]
            tcol[:n] = cb[sel]
            warr[lane, js] = wcol
            tgt_arr[lane, j] = tcol

    def wrap(stream, nch_s):
        out = np.zeros((P, nch_s * 8), np.int16)
        nbt = (nch_s + BATCH_CH - 1) // BATCH_CH
        for b in range(nbt):
            ch = min(BATCH_CH, nch_s - b * BATCH_CH)
            seg = stream[b * BATCH_CH * P: b * BATCH_CH * P + ch * P]
            w16 = seg.reshape(ch * 8, 16).T
            out[:, b * BATCH_CH * 8: b * BATCH_CH * 8 + ch * 8] = np.tile(w16, (8, 1))
        return out

    return wrap(il, nch_lo), wrap(ih, nch_hi), wlo, whi, tgt_arr


def _bf16(a):
    import ml_dtypes
    return a.astype(ml_dtypes.bfloat16)


def kernel(x, blocks, edge_weights, source, target, edge_type):
    from concourse.bass_utils import run_bass_kernel_spmd

    x = np.asarray(x, np.float32)
    blocks = np.asarray(blocks, np.float32)
    edge_weights = np.asarray(edge_weights, np.float32)
    source = np.asarray(source, np.int64)
    target = np.asarray(target, np.int64)
    edge_type = np.asarray(edge_type, np.int64)

    n, d = x.shape
    assert n == N_NODES and d == D

    xp = np.zeros((N_PAD, 2 * D), np.float32)
    xp[:n, :D] = x
    xp[:n, D:] = x
    xd = _bf16(xp)

    iota = _bf16(np.broadcast_to(np.arange(NCOL, dtype=np.float32), (P, NCOL)).copy())

    bs = D // blocks.shape[1]
    wtab = np.zeros((D, R * D), np.float32)
    for r in range(R):
        for b in range(blocks.shape[1]):
            wtab[b * bs:(b + 1) * bs, r * D + b * bs:r * D + (b + 1) * bs] = blocks[r, b]
    wtab = _bf16(wtab)

    deg = np.bincount(target, minlength=N_PAD)
    win_of, off_of = _pack_windows(deg)

    ewin = win_of[target]
    eoff = off_of[target]
    ecore = np.minimum(ewin // WPC, R - 1)
    per_core = []
    c_tot_min = 1
    max_ml = max_mh = 0
    for c in range(R):
        m = ecore == c
        src_c = source[m]
        bidx = (ewin[m] - c * WPC).astype(np.int64)
        col = (edge_type[m] * TW + eoff[m]).astype(np.float32)
        wgt_c = edge_weights[m]
        order = np.argsort(bidx, kind="stable")
        src_c, bidx, col, wgt_c = (a[order] for a in (src_c, bidx, col, wgt_c))
        T = np.bincount(bidx, minlength=WPC)
        ml = np.bincount(bidx[src_c < HI_START], minlength=WPC)
        mh = np.bincount(bidx[src_c >= LO_END], minlength=WPC)
        c_tot_min = max(c_tot_min, int(-(-T.max() // P)))
        max_ml = max(max_ml, int(ml.max()))
        max_mh = max(max_mh, int(mh.max()))
        per_core.append((src_c, bidx, col, wgt_c))

    c_lo = max(1, int(-(-max_ml // P)))
    c_hi = max(1, c_tot_min - c_lo)
    while c_hi * P < max_mh:
        c_hi += 1

    key = (c_lo, c_hi)
    if key not in _cache:
        _cache[key] = _build_program(c_lo, c_hi)
    nc = _cache[key]

    in_maps = []
    for c in range(R):
        src_c, bidx, col, wgt_c = per_core[c]
        il, ih, wlo, whi, tgt_arr = _prep_core(src_c, bidx, col, wgt_c, c_lo, c_hi)
        in_maps.append({
            "xd": xd, "il": il, "ih": ih,
            "wlo": _bf16(wlo), "whi": _bf16(whi), "tgt": _bf16(tgt_arr),
            "iota": iota, "wtab": wtab,
        })

    res = run_bass_kernel_spmd(nc, in_maps, core_ids=list(range(R)))

    out_perm = np.concatenate([res.results[c]["outT"] for c in range(R)], axis=1)
    t = np.arange(N_NODES)
    return np.ascontiguousarray(out_perm[:, win_of[t] * TW + off_of[t]].T)
